# revision 7
# baseline (speedup 1.0000x reference)
"""Trainium2 Bass kernel for nn_CP_Based (CP-decomposition interaction layer).

Math (full problem):
    t[b,f,r,u] = sum_d X[b,f,d] * K[d,r,f,u]      (B=1024, F=64, D=4, R=32, U=128)
    had[b,r,u] = prod_f t[b,f,r,u]
    out[b,u]   = sum_r had[b,r,u]

Strategy:
  * Shard batch across 8 cores (B_loc = 128 = one partition tile).
  * Feature-pairing (host-side weight repack): for pair p=(2p, 2p+1),
        t2[b,p,r,u] = t[b,2p,r,u] * t[b,2p+1,r,u]
                    = sum_{d,e} (X[b,2p,d]*X[b,2p+1,e]) * (K[d,r,2p,u]*K[e,r,2p+1,u])
                    = sum_{de} X2[b,p,de] * K2[de,r,p,u]
    i.e. one K=16 matmul per pair instead of two K=4 matmuls — halves the
    elementwise hadamard work (32 factors instead of 64).
  * PE: 4 pairs run concurrently via 4x row tiling (tile_position=(32i,0)),
    one PSUM bank each per 512-column chunk of (r,u).
  * DVE: binary product tree per chunk; running product P[b, r*u] in SBUF.
  * Final sum over r: strided tensor_reduce.

Host prep is pure input repacking (outer products of the small inputs,
~2.6M mults vs ~1.3G on device).
"""

import numpy as np

B, F, D, R, U = 1024, 64, 4, 32, 128
NCORES = 8
BLOC = B // NCORES          # 128 batch rows per core
NPAIR = F // 2              # 32 feature pairs
NGRP = NPAIR // 4           # 8 groups of 4 row-tiled pairs
DE = D * D                  # 16 = contraction dim per pair
RU = R * U                  # 4096
CHUNK = 512                 # psum bank = 512 fp32
NCHUNK = RU // CHUNK        # 8

_cached = {}


def _build_nc(n_rep=1):
    import concourse.bass as bass
    import concourse.mybir as mybir
    import concourse.tile as tile
    from concourse import bacc

    fp32 = mybir.dt.float32
    nc = bacc.Bacc("TRN2", target_bir_lowering=False, debug=False)

    x2t_d = nc.dram_tensor("x2t", [128, NGRP * BLOC], fp32, kind="ExternalInput").ap()
    kr2_d = nc.dram_tensor("kr2", [NGRP, 128, RU], fp32, kind="ExternalInput").ap()
    out_d = nc.dram_tensor("out", [BLOC, U], fp32, kind="ExternalOutput").ap()

    with tile.TileContext(nc) as tc:
        with (
            tc.tile_pool(name="const", bufs=1) as const_pool,
            tc.tile_pool(name="kt", bufs=2) as kpool,
            tc.tile_pool(name="prod", bufs=1) as ppool,

            tc.tile_pool(name="outp", bufs=1) as opool,
            tc.tile_pool(name="ps", bufs=8, space="PSUM") as pspool,
        ):
            x2t = const_pool.tile([128, NGRP * BLOC], fp32)
            nc.sync.dma_start(x2t[:], x2t_d[:])

            P = ppool.tile([128, RU], fp32)

            def body():
                for g in range(NGRP):
                    kt = kpool.tile([128, RU], fp32, tag="kt")
                    nc.sync.dma_start(kt[:], kr2_d[g])
                    for c in range(NCHUNK):
                        sl = slice(c * CHUNK, (c + 1) * CHUNK)
                        ps = []
                        for i in range(4):
                            pst = pspool.tile([128, CHUNK], fp32, tag="ps")
                            nc.tensor.matmul(
                                pst[:],
                                x2t[32 * i : 32 * i + DE, g * BLOC : (g + 1) * BLOC],
                                kt[32 * i : 32 * i + DE, sl],
                                start=True,
                                stop=True,
                                tile_position=(32 * i, 0),
                            )
                            ps.append(pst)
                        # DVE reads at most one PSUM operand per op: chain the
                        # running product through SBUF. g=0 init via ScalarE.
                        if g == 0:
                            nc.scalar.copy(P[:, sl], ps[0][:])
                        else:
                            nc.vector.tensor_mul(P[:, sl], P[:, sl], ps[0][:])
                        for i in range(1, 4):
                            nc.vector.tensor_mul(P[:, sl], P[:, sl], ps[i][:])

            if n_rep == 1:
                body()
            else:
                # benchmarking mode: repeat the (idempotent) body on-device
                with tc.For_i(0, n_rep, 1):
                    body()

            osum = opool.tile([BLOC, U], fp32)
            nc.vector.tensor_reduce(
                osum[:],
                P[:].rearrange("p (r u) -> p u r", r=R),
                axis=mybir.AxisListType.X,
                op=mybir.AluOpType.add,
            )
            nc.sync.dma_start(out_d[:], osum[:])

    nc.compile()
    return nc


def _host_prep(X, K):
    """Repack inputs: per-core X2 outer products + shared K2 outer products."""
    f32 = np.float32
    # K2: [de, r, p, u] -> packed [g, 128, r*u] with pair i at rows 32i..32i+15
    ka = K[:, :, 0::2, :]                        # [4, 32, 32, 128] (d, r, p, u)
    kb = K[:, :, 1::2, :]
    K2 = ka[:, None] * kb[None, :]               # [4, 4, 32, 32, 128] (d,e,r,p,u)
    K2 = K2.transpose(3, 0, 1, 2, 4).reshape(NPAIR, DE, RU)  # [p, de, r*u]
    kr2 = np.zeros((NGRP, 4, 32, RU), dtype=f32)
    kr2[:, :, :DE, :] = K2.reshape(NGRP, 4, DE, RU)
    kr2 = np.ascontiguousarray(kr2.reshape(NGRP, 128, RU))

    x2ts = []
    for c in range(NCORES):
        Xc = X[c * BLOC : (c + 1) * BLOC]        # [128, 64, 4]
        xa = Xc[:, 0::2, :]                      # [128, 32, 4] (b, p, d)
        xb = Xc[:, 1::2, :]
        X2 = xa[:, :, :, None] * xb[:, :, None, :]   # [b, p, d, e]
        A = X2.transpose(1, 2, 3, 0).reshape(NPAIR, DE, BLOC)  # [p, de, b]
        x2t = np.zeros((4, 32, NGRP, BLOC), dtype=f32)
        x2t[:, :DE] = A.reshape(NGRP, 4, DE, BLOC).transpose(1, 2, 0, 3)
        x2ts.append(np.ascontiguousarray(x2t.reshape(128, NGRP * BLOC)))
    return x2ts, kr2


def kernel(**inputs):
    from concourse.bass_utils import run_bass_kernel_spmd

    X = np.asarray(inputs["X"], dtype=np.float32)
    K = np.asarray(inputs["kernel"], dtype=np.float32)
    assert X.shape == (B, F, D) and K.shape == (D, R, F, U)

    if "nc" not in _cached:
        _cached["nc"] = _build_nc()
    nc = _cached["nc"]

    x2ts, kr2 = _host_prep(X, K)
    in_maps = [{"x2t": x2ts[c], "kr2": kr2} for c in range(NCORES)]
    res = run_bass_kernel_spmd(nc, in_maps, core_ids=list(range(NCORES)))
    return np.concatenate([res.results[c]["out"] for c in range(NCORES)], axis=0)


# revision 12
# speedup vs baseline: 5.1989x; 5.1989x over previous
"""Trainium2 Bass kernel for nn_CP_Based (CP-decomposition interaction layer).

Math (full problem):
    t[b,f,r,u] = sum_d X[b,f,d] * K[d,r,f,u]      (B=1024, F=64, D=4, R=32, U=128)
    had[b,r,u] = prod_f t[b,f,r,u]
    out[b,u]   = sum_r had[b,r,u]

Strategy:
  * Shard batch across 8 cores (B_loc = 128 = one partition tile).
  * Feature-tripling (host-side weight repack): for a triple (f0,f1,f2),
        t3 = t[.,f0,.] * t[.,f1,.] * t[.,f2,.]
           = sum_{d3=0..63} X3[b,j,d3] * K3[d3,r,j,u]
    with X3/K3 outer products of the per-feature slices. One K=64 matmul per
    triple replaces three K=4 matmuls AND cuts the elementwise hadamard from
    63 to 21 multiplies per output element (the DVE is the bottleneck engine:
    fp32 tensor_tensor runs at 1 elem/cycle/partition @ 0.96 GHz).
    64 = 21*3 + 1: factor 21 is the lone feature 63, zero-padded to K=64.
  * PE: 2 factors run concurrently via row tiling (tile_position=(64s,0)),
    each filling a [128,1024] 2-bank psum tile per (r,u) chunk.
  * DVE: running product P[b, r*u] *= psum factor chunks (one PSUM operand
    per op is a HW limit). ScalarE initializes P for the first factor.
  * Final sum over r: strided tensor_reduce.

Host prep is pure input repacking (outer products of the small inputs,
~12M mults vs ~1.3G MACs + 270M multiplies on device).
"""

import numpy as np

B, F, D, R, U = 1024, 64, 4, 32, 128
NCORES = 8
BLOC = B // NCORES          # 128 batch rows per core
NFAC = 22                   # 21 triples + 1 padded single
NGRP = NFAC // 2            # 11 groups of 2 row-tiled factors
D3 = 64                     # contraction dim per triple (4^3)
RU = R * U                  # 4096
CHUNK = 1024                # 2 psum banks per factor-chunk
NCHUNK = RU // CHUNK        # 4

_cached = {}


def _build_nc(n_rep=1, chunk=CHUNK):
    import concourse.bass as bass
    import concourse.mybir as mybir
    import concourse.tile as tile
    from concourse import bacc

    nch = RU // chunk
    nps = 8 // (chunk // 512)  # psum tiles to fill all 8 banks
    fp32 = mybir.dt.float32
    nc = bacc.Bacc("TRN2", target_bir_lowering=False, debug=False)

    xt_d = nc.dram_tensor("xt", [128, NGRP * BLOC], fp32, kind="ExternalInput").ap()
    kr_d = nc.dram_tensor("kr", [NGRP, 128, RU], fp32, kind="ExternalInput").ap()
    out_d = nc.dram_tensor("out", [BLOC, U], fp32, kind="ExternalOutput").ap()

    with tile.TileContext(nc) as tc:
        with (
            tc.tile_pool(name="const", bufs=1) as const_pool,
            tc.tile_pool(name="kt", bufs=3) as kpool,
            tc.tile_pool(name="prod", bufs=1) as ppool,
            tc.tile_pool(name="outp", bufs=1) as opool,
            tc.tile_pool(name="ps", bufs=nps, space="PSUM") as pspool,
        ):
            xt = const_pool.tile([128, NGRP * BLOC], fp32)
            nc.sync.dma_start(xt[:], xt_d[:])

            P = ppool.tile([128, RU], fp32)

            def body():
                for m in range(NGRP):
                    kt = kpool.tile([128, RU], fp32, tag="kt")
                    nc.sync.dma_start(kt[:], kr_d[m])
                    for c in range(nch):
                        sl = slice(c * chunk, (c + 1) * chunk)
                        ps = []
                        for s in range(2):
                            pst = pspool.tile([128, chunk], fp32, tag="ps")
                            for h in range(chunk // 512):
                                hs = slice(h * 512, (h + 1) * 512)
                                nc.tensor.matmul(
                                    pst[:, hs],
                                    xt[64 * s : 64 * s + D3, m * BLOC : (m + 1) * BLOC],
                                    kt[64 * s : 64 * s + D3, c * chunk + h * 512 : c * chunk + (h + 1) * 512],
                                    start=True,
                                    stop=True,
                                    tile_position=(64 * s, 0),
                                )
                            ps.append(pst)
                        # DVE reads at most one PSUM operand per op: chain the
                        # running product through SBUF. Init via ScalarE copy.
                        if m == 0:
                            nc.scalar.copy(P[:, sl], ps[0][:])
                        else:
                            nc.vector.tensor_mul(P[:, sl], P[:, sl], ps[0][:])
                        nc.vector.tensor_mul(P[:, sl], P[:, sl], ps[1][:])

            if n_rep == 1:
                body()
            else:
                # benchmarking mode: repeat the (idempotent) body on-device
                with tc.For_i(0, n_rep, 1):
                    body()

            osum = opool.tile([BLOC, U], fp32)
            nc.vector.tensor_reduce(
                osum[:],
                P[:].rearrange("p (r u) -> p u r", r=R),
                axis=mybir.AxisListType.X,
                op=mybir.AluOpType.add,
            )
            nc.sync.dma_start(out_d[:], osum[:])

    nc.compile()
    return nc


def _host_prep(X, K):
    """Repack inputs: per-core X3 outer products + shared K3 outer products.

    Factor j < 21 covers features (3j, 3j+1, 3j+2) with contraction index
    d3 = 16*d0 + 4*d1 + d2; factor 21 is feature 63 (d3 = d, rest zero).
    Packed layouts match SBUF tiles directly:
      kr[m, row, r*U+u]: row = 64*s + d3 holds factor (2m+s).
      xt[row, m*BLOC+b]: same row convention.
    """
    f32 = np.float32
    NT = 21
    fa = [3 * j for j in range(NT)]

    # K3 [j, d3, r*u]
    ka = K[:, :, [3 * j for j in range(NT)], :]      # [4, 32, 21, 128] (d,r,j,u)
    kb = K[:, :, [3 * j + 1 for j in range(NT)], :]
    kc = K[:, :, [3 * j + 2 for j in range(NT)], :]
    K3 = (
        ka[:, None, None] * kb[None, :, None] * kc[None, None, :]
    )                                                # [4,4,4,32,21,128] (d0,d1,d2,r,j,u)
    K3 = K3.transpose(4, 0, 1, 2, 3, 5).reshape(NT, D3, RU)  # [j, d3, r*u]
    K3f = np.zeros((NFAC, D3, RU), dtype=f32)
    K3f[:NT] = K3
    K3f[NT, :D, :] = K[:, :, 63, :].reshape(D, RU)   # lone feature 63
    kr = np.ascontiguousarray(
        K3f.reshape(NGRP, 2, D3, RU).reshape(NGRP, 128, RU)
    )

    # X3 per core [row, m*BLOC+b]
    xts = []
    for c in range(NCORES):
        Xc = X[c * BLOC : (c + 1) * BLOC]            # [128, 64, 4] (b, f, d)
        xa = Xc[:, [3 * j for j in range(NT)], :]    # [b, j, 4]
        xb = Xc[:, [3 * j + 1 for j in range(NT)], :]
        xc = Xc[:, [3 * j + 2 for j in range(NT)], :]
        X3 = (
            xa[:, :, :, None, None] * xb[:, :, None, :, None] * xc[:, :, None, None, :]
        )                                            # [b, j, 4, 4, 4]
        X3 = X3.reshape(BLOC, NT, D3)
        X3f = np.zeros((BLOC, NFAC, D3), dtype=f32)
        X3f[:, :NT] = X3
        X3f[:, NT, :D] = Xc[:, 63, :]
        xt = X3f.transpose(1, 2, 0).reshape(NGRP, 128, BLOC)  # [m, row, b]
        xts.append(np.ascontiguousarray(xt.transpose(1, 0, 2).reshape(128, NGRP * BLOC)))
    return xts, kr


def kernel(**inputs):
    from concourse.bass_utils import run_bass_kernel_spmd

    X = np.asarray(inputs["X"], dtype=np.float32)
    K = np.asarray(inputs["kernel"], dtype=np.float32)
    assert X.shape == (B, F, D) and K.shape == (D, R, F, U)

    if "nc" not in _cached:
        _cached["nc"] = _build_nc()
    nc = _cached["nc"]

    xts, kr = _host_prep(X, K)
    in_maps = [{"xt": xts[c], "kr": kr} for c in range(NCORES)]
    res = run_bass_kernel_spmd(nc, in_maps, core_ids=list(range(NCORES)))
    return np.concatenate([res.results[c]["out"] for c in range(NCORES)], axis=0)
